# revision 3
# baseline (speedup 1.0000x reference)
"""FCOS head (nn_FCOS_73787538145418) Trainium2 Bass kernel.

Sharding: data-parallel, one image per NeuronCore (B=8 across 8 cores),
weights replicated. Each core runs the identical SPMD NEFF over its image:
for each FPN level (p3 64x64, p4 32x32, p5 16x16):
  - two 4-layer 3x3 conv stems (cls / box), 256->256 + ReLU
  - prediction convs: cls (20ch), box+ctr (5ch, packed)
3x3 'same' convs are computed as 18 PSUM-accumulated matmuls (2 ci chunks x
9 taps, K=128 each) over spatially padded SBUF buffers; dtype float32r
(full-rate fp32 path on the PE).

Output on-device is [25, 5376] channel-major per core; host transposes and
stacks to (8, 5376, 25).
"""
import sys

if '/opt/trn_rl_repo' not in sys.path:
    sys.path.insert(0, '/opt/trn_rl_repo')

import numpy as np

import concourse.bass as bass
import concourse.mybir as mybir
from concourse import bacc
import concourse.tile as tile
from concourse.bass_utils import run_bass_kernel_spmd

P = 128
NCH = 2                 # 256 channels = 2 chunks of 128
C = 256
NL = 4                  # stem depth
LEVELS = [(64, 64, 8), (32, 32, 16), (16, 16, 16)]   # (H, W, rows-per-tile)
NPIX_TOTAL = sum(h * w for h, w, _ in LEVELS)        # 5376
F32R = mybir.dt.float32r
F32 = mybir.dt.float32

_cached = {}
_run_opts = {}   # extra kwargs for run_bass_kernel_spmd (test harness: trace)
_last = {}       # last BassKernelResults (test harness reads exec_time_ns)


def _conv_layer(nc, psum_pool, wt, src, dst, bias_ap, H, W, R, relu=True):
    """One 3x3 same conv 256->256 (+bias, +relu) between padded SBUF views.

    src/dst: [P, NCH, H+2, W+2] fp32r padded views. wt: [P, NCH, NCH, 9, P].
    """
    n_tiles = H // R
    for o in range(NCH):
        pss = [
            psum_pool.tile([P, R, W], F32, tag="ps", name=f"ps{o}_{it}")
            for it in range(n_tiles)
        ]
        k = 0
        for c in range(NCH):
            for ky in range(3):
                for kx in range(3):
                    lhsT = wt[:, c, o, ky * 3 + kx, :]
                    for it in range(n_tiles):
                        r0 = it * R
                        rhs = src[:, c, r0 + ky:r0 + ky + R, kx:kx + W]
                        nc.tensor.matmul(pss[it][:], lhsT, rhs,
                                         start=(k == 0), stop=(k == 17))
                    k += 1
        for it in range(n_tiles):
            r0 = it * R
            nc.scalar.activation(dst[:, o, r0 + 1:r0 + 1 + R, 1:W + 1], pss[it][:],
                                 mybir.ActivationFunctionType.Relu,
                                 bias=bias_ap[:, o])


def _pred_conv(nc, psum_pool, stage_pool, wt, bias_sb, src, out_d,
               n_out, out_row0, H, W, R, pix_base):
    """3x3 conv 256->n_out from padded tower `src`, + bias, written to
    out_d[out_row0:out_row0+n_out, pix_base:...] channel-major."""
    n_tiles = H // R
    for it in range(n_tiles):
        r0 = it * R
        ps = psum_pool.tile([P, R, W], F32, tag="ps", name=f"pp{it}")
        k = 0
        for c in range(NCH):
            for ky in range(3):
                for kx in range(3):
                    lhsT = wt[:, c, ky * 3 + kx, :]
                    rhs = src[:, c, r0 + ky:r0 + ky + R, kx:kx + W]
                    nc.tensor.matmul(ps[:n_out], lhsT, rhs,
                                     start=(k == 0), stop=(k == 17))
                    k += 1
        st = stage_pool.tile([32, R * W], F32, tag="st", name=f"st{it}")
        nc.vector.tensor_tensor(
            st[:n_out], ps[:n_out].rearrange("p r w -> p (r w)"),
            bias_sb[:n_out].to_broadcast([n_out, R * W]),
            mybir.AluOpType.add)
        nc.sync.dma_start(
            out_d[out_row0:out_row0 + n_out,
                  pix_base + r0 * W: pix_base + (r0 + R) * W],
            st[:n_out])


def _zero_ring(nc, buf, H, W):
    """Zero the 1-px padding ring of a [P, NCH, H+2, W+2] fp32r view."""
    v = buf.bitcast(F32)
    for c in range(NCH):
        nc.vector.memset(v[:, c, 0, :], 0.0)            # top row
        nc.vector.memset(v[:, c, H + 1, :], 0.0)        # bottom row
        nc.vector.memset(v[:, c, 1:H + 1, 0], 0.0)      # left col
        nc.vector.memset(v[:, c, 1:H + 1, W + 1], 0.0)  # right col


def _build():
    nc = bacc.Bacc("TRN2", target_bir_lowering=False, debug=False,
                   num_devices=8)

    x_d = [nc.dram_tensor(f"x{i}", (NCH, P, h, w), F32R, kind="ExternalInput")
           for i, (h, w, _) in enumerate(LEVELS)]
    sw_d = nc.dram_tensor("sw", (2, NL, P, NCH, NCH, 9, P), F32R,
                          kind="ExternalInput")
    sb_d = nc.dram_tensor("sb", (2, NL, NCH, P, 1), F32, kind="ExternalInput")
    pwc_d = nc.dram_tensor("pwc", (P, NCH, 9, 20), F32R, kind="ExternalInput")
    pwb_d = nc.dram_tensor("pwb", (P, NCH, 9, 5), F32R, kind="ExternalInput")
    pbc_d = nc.dram_tensor("pbc", (20, 1), F32, kind="ExternalInput")
    pbb_d = nc.dram_tensor("pbb", (5, 1), F32, kind="ExternalInput")
    out_d = nc.dram_tensor("out", (25, NPIX_TOTAL), F32, kind="ExternalOutput")

    PBUF = NCH * 66 * 66   # flat fp32r elems per partition per pad buffer

    with tile.TileContext(nc) as tc:
        with (
            tc.tile_pool(name="resident", bufs=1) as res_pool,
            tc.tile_pool(name="wts", bufs=2) as wts_pool,
            tc.tile_pool(name="psum", bufs=8, space="PSUM") as psum_pool,
            tc.tile_pool(name="stage", bufs=4) as stage_pool,
        ):
            pads = [res_pool.tile([P, PBUF], F32R, name=f"pad{i}")
                    for i in range(3)]
            sbias = res_pool.tile([P, 2, NL, NCH, 1], F32, name="sbias")
            nc.sync.dma_start(
                sbias[:],
                sb_d[:].rearrange("s l a p o -> p (s l a o)")
                       .rearrange("p (s l a o) -> p s l a o", s=2, l=NL, a=NCH))
            pwc = res_pool.tile([P, NCH, 9, 20], F32R, name="pwc")
            pwb = res_pool.tile([P, NCH, 9, 5], F32R, name="pwb")
            nc.sync.dma_start(pwc[:], pwc_d[:])
            nc.sync.dma_start(pwb[:], pwb_d[:])
            pbc = res_pool.tile([32, 1], F32, name="pbc")
            pbb = res_pool.tile([32, 1], F32, name="pbb")
            nc.sync.dma_start(pbc[:20], pbc_d[:])
            nc.sync.dma_start(pbb[:5], pbb_d[:])

            pix_base = 0
            for li, (H, W, R) in enumerate(LEVELS):
                HP, WP = H + 2, W + 2
                n_el = NCH * HP * WP
                views = [
                    pads[i][:, :n_el].rearrange("p (c h w) -> p c h w",
                                                c=NCH, h=HP, w=WP)
                    for i in range(3)
                ]
                feat = views[0]
                for v in views:
                    _zero_ring(nc, v, H, W)
                for c in range(NCH):
                    nc.sync.dma_start(feat[:, c, 1:H + 1, 1:W + 1], x_d[li][c])

                for s in range(2):          # 0 = cls stem, 1 = box stem
                    src = feat
                    for l in range(NL):
                        wt = wts_pool.tile([P, NCH, NCH, 9, P], F32R,
                                           tag="w", name=f"w{li}_{s}_{l}")
                        nc.sync.dma_start(wt[:], sw_d[s, l])
                        dst = views[1 + (l % 2)]
                        _conv_layer(nc, psum_pool, wt, src, dst,
                                    sbias[:, s, l], H, W, R)
                        src = dst
                    # src is now the tower output (views[2] since NL=4)
                    if s == 0:
                        _pred_conv(nc, psum_pool, stage_pool, pwc, pbc,
                                   src, out_d, 20, 0, H, W, R, pix_base)
                    else:
                        _pred_conv(nc, psum_pool, stage_pool, pwb, pbb,
                                   src, out_d, 5, 20, H, W, R, pix_base)
                pix_base += H * W

    nc.compile()
    return nc


def _pack_stem_w(wcls, wbox):
    # [s, l, co, ci, ky, kx] -> [s, l, cip, cic, coc, tap, cop]
    w = np.stack([wcls, wbox]).reshape(2, NL, NCH, P, NCH, P, 3, 3)
    w = w.transpose(0, 1, 5, 4, 2, 6, 7, 3)
    return np.ascontiguousarray(w.reshape(2, NL, P, NCH, NCH, 9, P),
                                dtype=np.float32)


def _pack_pred_w(w):
    # [co, ci, ky, kx] -> [cip, cic, tap, co]
    n = w.shape[0]
    w = w.reshape(n, NCH, P, 3, 3).transpose(2, 1, 3, 4, 0)
    return np.ascontiguousarray(w.reshape(P, NCH, 9, n), dtype=np.float32)


def kernel(p3, p4, p5, stem_cls_w, stem_cls_b, stem_box_w, stem_box_b,
           pred_cls_w, pred_cls_b, pred_box_w, pred_box_b,
           pred_ctr_w, pred_ctr_b):
    if 'nc' not in _cached:
        _cached['nc'] = _build()
    nc = _cached['nc']

    B = p3.shape[0]
    sw = _pack_stem_w(np.asarray(stem_cls_w), np.asarray(stem_box_w))
    sb = np.ascontiguousarray(
        np.stack([stem_cls_b, stem_box_b]).reshape(2, NL, NCH, P, 1),
        dtype=np.float32)
    pwc = _pack_pred_w(np.asarray(pred_cls_w))
    pwb = _pack_pred_w(np.concatenate([pred_box_w, pred_ctr_w], axis=0))
    pbc = np.asarray(pred_cls_b, np.float32).reshape(20, 1)
    pbb = np.concatenate([pred_box_b, pred_ctr_b]).astype(np.float32).reshape(5, 1)

    shared = {"sw": sw, "sb": sb, "pwc": pwc, "pwb": pwb,
              "pbc": pbc, "pbb": pbb}
    xs = [np.asarray(p3, np.float32), np.asarray(p4, np.float32),
          np.asarray(p5, np.float32)]
    in_maps = []
    for b in range(B):
        m = dict(shared)
        for i, x in enumerate(xs):
            m[f"x{i}"] = np.ascontiguousarray(
                x[b].reshape(NCH, P, x.shape[2], x.shape[3]))
        in_maps.append(m)

    res = run_bass_kernel_spmd(nc, in_maps, core_ids=list(range(B)),
                               **_run_opts)
    _last['res'] = res
    out = np.stack([r["out"].T for r in res.results])
    return np.ascontiguousarray(out, dtype=np.float32)


# revision 7
# speedup vs baseline: 1.0107x; 1.0107x over previous
"""FCOS head (nn_FCOS_73787538145418) Trainium2 Bass kernel.

Sharding: data-parallel, one image per NeuronCore (B=8 across 8 cores),
weights replicated. Each core runs the identical SPMD NEFF over its image.

Per level (p3 64x64, p4 32x32, p5 16x16): two 4-layer 3x3 conv stems
(cls/box, 256ch + ReLU), then prediction convs (cls 20ch; box+ctr 5ch).
3x3 'same' convs = 18 PSUM-accumulated matmuls (2 ci chunks x 9 taps,
K=128) over spatially padded SBUF buffers, dtype float32r (full-rate fp32
on the PE). Levels p4+p5 run as one fused pass sharing stem-weight DMAs.
The two prediction convs are column-tiled into PE col-groups 0 and 1 and
run concurrently. Output is [25, 5376] channel-major per core; the host
transposes and stacks to (8, 5376, 25).
"""
import sys

if '/opt/trn_rl_repo' not in sys.path:
    sys.path.insert(0, '/opt/trn_rl_repo')

import numpy as np

import concourse.bass as bass
import concourse.mybir as mybir
from concourse import bacc
import concourse.tile as tile
from concourse.bass_utils import run_bass_kernel_spmd

P = 128
NCH = 2                 # 256 channels = 2 chunks of 128
C = 256
NL = 4                  # stem depth
# (H, W, rows-per-pixel-tile, flat-offset of feat buffer, pixel base)
LEVELS = [
    dict(H=64, W=64, R=8, pix=0),
    dict(H=32, W=32, R=16, pix=4096),
    dict(H=16, W=16, R=16, pix=5120),
]
NPIX_TOTAL = 5376
F32R = mybir.dt.float32r
F32 = mybir.dt.float32

_cached = {}
_run_opts = {}   # extra kwargs for run_bass_kernel_spmd (test harness: trace)
_last = {}       # last BassKernelResults (test harness reads exec_time_ns)


def _pad_view(flat_tile, off, H, W):
    n = NCH * (H + 2) * (W + 2)
    return flat_tile[:, off:off + n].rearrange(
        "p (c h w) -> p c h w", c=NCH, h=H + 2, w=W + 2)


def _zero_ring(nc, v, H, W):
    f = v.bitcast(F32)
    for c in range(NCH):
        nc.vector.memset(f[:, c, 0, :], 0.0)
        nc.vector.memset(f[:, c, H + 1, :], 0.0)
        nc.vector.memset(f[:, c, 1:H + 1, 0], 0.0)
        nc.vector.memset(f[:, c, 1:H + 1, W + 1], 0.0)


def _conv_layer(nc, psum_pool, wt, src, dst, bias_ap, H, W, R, tag):
    """3x3 same conv 256->256 + bias + relu between padded fp32r views."""
    n_tiles = H // R
    for o in range(NCH):
        pss = [
            psum_pool.tile([P, R, W], F32, tag="ps", name=f"ps_{tag}_{o}_{it}")
            for it in range(n_tiles)
        ]
        k = 0
        for c in range(NCH):
            for ky in range(3):
                for kx in range(3):
                    lhsT = wt[:, c, o, ky * 3 + kx, :]
                    for it in range(n_tiles):
                        r0 = it * R
                        rhs = src[:, c, r0 + ky:r0 + ky + R, kx:kx + W]
                        nc.tensor.matmul(pss[it][:], lhsT, rhs,
                                         start=(k == 0), stop=(k == 17))
                    k += 1
        for it in range(n_tiles):
            r0 = it * R
            nc.scalar.activation(dst[:, o, r0 + 1:r0 + 1 + R, 1:W + 1],
                                 pss[it][:],
                                 mybir.ActivationFunctionType.Relu,
                                 bias=bias_ap[:, o])


def _preds(nc, psum_pool, stage_pool, pwc, pwb, pbc, pbb,
           cls_tower, box_tower, out_d, H, W, R, pix_base, tag):
    """cls (20ch) and box+ctr (5ch) 3x3 prediction convs (PSUM base 0)."""
    n_tiles = H // R
    for it in range(n_tiles):
        r0 = it * R
        ps1 = psum_pool.tile([P, R, W], F32, tag="ps", name=f"pc_{tag}_{it}")
        ps2 = psum_pool.tile([P, R, W], F32, tag="ps", name=f"pb_{tag}_{it}")
        k = 0
        for c in range(NCH):
            for ky in range(3):
                for kx in range(3):
                    t = ky * 3 + kx
                    rc = cls_tower[:, c, r0 + ky:r0 + ky + R, kx:kx + W]
                    rb = box_tower[:, c, r0 + ky:r0 + ky + R, kx:kx + W]
                    nc.tensor.matmul(ps1[0:20], pwc[:, c, t, :], rc,
                                     start=(k == 0), stop=(k == 17))
                    nc.tensor.matmul(ps2[0:5], pwb[:, c, t, :], rb,
                                     start=(k == 0), stop=(k == 17))
                    k += 1
        st = stage_pool.tile([32, R * W], F32, tag="st", name=f"st_{tag}_{it}")
        st2 = stage_pool.tile([32, R * W], F32, tag="st", name=f"s2_{tag}_{it}")
        nc.vector.tensor_tensor(
            st[0:20], ps1[0:20].rearrange("p r w -> p (r w)"),
            pbc[:20].to_broadcast([20, R * W]), mybir.AluOpType.add)
        nc.vector.tensor_tensor(
            st2[0:5], ps2[0:5].rearrange("p r w -> p (r w)"),
            pbb[:5].to_broadcast([5, R * W]), mybir.AluOpType.add)
        c0 = pix_base + r0 * W
        nc.sync.dma_start(out_d[0:20, c0:c0 + R * W], st[0:20])
        nc.sync.dma_start(out_d[20:25, c0:c0 + R * W], st2[0:5])


# Buffer rotation (3 pad buffers v0=feat, v1, v2):
#   cls stem: v0->v1->v2->v1->v2   (cls tower = v2)
#   box stem: v0->v1->v0->v1->v0   (box tower = v0; feat dead after box l1)
_CLS_CHAIN = [(0, 1), (1, 2), (2, 1), (1, 2)]
_BOX_CHAIN = [(0, 1), (1, 0), (0, 1), (1, 0)]


def _pass(nc, psum_pool, wts_pool, stage_pool, lvl_views, sw_d, sbias,
          pwc, pwb, pbc, pbb, out_d, tag):
    """One full pass (stems + preds) over a list of levels sharing weight DMAs.

    lvl_views: list of (views[3], H, W, R, pix_base)."""
    for s in range(2):
        chain = _CLS_CHAIN if s == 0 else _BOX_CHAIN
        for l in range(NL):
            wt = wts_pool.tile([P, NCH, NCH, 9, P], F32R,
                               tag="w", name=f"w_{tag}_{s}_{l}")
            nc.sync.dma_start(wt[:], sw_d[s, l])
            si, di = chain[l]
            for vi, (views, H, W, R, pix) in enumerate(lvl_views):
                _conv_layer(nc, psum_pool, wt, views[si], views[di],
                            sbias[:, s, l], H, W, R, f"{tag}{vi}_{s}{l}")
    for vi, (views, H, W, R, pix) in enumerate(lvl_views):
        _preds(nc, psum_pool, stage_pool, pwc, pwb, pbc, pbb,
               views[2], views[0], out_d, H, W, R, pix, f"{tag}{vi}")


def _build():
    nc = bacc.Bacc("TRN2", target_bir_lowering=False, debug=False,
                   num_devices=8)

    x_d = [nc.dram_tensor(f"x{i}", (NCH, P, lv['H'], lv['W']), F32R,
                          kind="ExternalInput")
           for i, lv in enumerate(LEVELS)]
    sw_d = nc.dram_tensor("sw", (2, NL, P, NCH, NCH, 9, P), F32R,
                          kind="ExternalInput")
    sb_d = nc.dram_tensor("sb", (2, NL, NCH, P, 1), F32, kind="ExternalInput")
    pwc_d = nc.dram_tensor("pwc", (P, NCH, 9, 20), F32R, kind="ExternalInput")
    pwb_d = nc.dram_tensor("pwb", (P, NCH, 9, 5), F32R, kind="ExternalInput")
    pbc_d = nc.dram_tensor("pbc", (20, 1), F32, kind="ExternalInput")
    pbb_d = nc.dram_tensor("pbb", (5, 1), F32, kind="ExternalInput")
    out_d = nc.dram_tensor("out", (25, NPIX_TOTAL), F32, kind="ExternalOutput")

    N3 = NCH * 66 * 66            # 8712: p3 padded elems/partition
    N4 = NCH * 34 * 34            # 2312
    N5 = NCH * 18 * 18            # 648
    # pad0 additionally holds the p4/p5 feat regions (prefetched at start)
    PAD0 = N3 + N4 + N5

    with tile.TileContext(nc) as tc:
        with (
            tc.tile_pool(name="resident", bufs=1) as res_pool,
            tc.tile_pool(name="wts", bufs=2) as wts_pool,
            tc.tile_pool(name="psum", bufs=8, space="PSUM") as psum_pool,
            tc.tile_pool(name="stage", bufs=4) as stage_pool,
        ):
            pad0 = res_pool.tile([P, PAD0], F32R, name="pad0")
            pad1 = res_pool.tile([P, N3], F32R, name="pad1")
            pad2 = res_pool.tile([P, N3], F32R, name="pad2")

            sbias = res_pool.tile([P, 2, NL, NCH, 1], F32, name="sbias")
            nc.sync.dma_start(
                sbias[:],
                sb_d[:].rearrange("s l a p o -> p (s l a o)")
                       .rearrange("p (s l a o) -> p s l a o",
                                  s=2, l=NL, a=NCH))
            pwc = res_pool.tile([P, NCH, 9, 20], F32R, name="pwc")
            pwb = res_pool.tile([P, NCH, 9, 5], F32R, name="pwb")
            nc.sync.dma_start(pwc[:], pwc_d[:])
            nc.sync.dma_start(pwb[:], pwb_d[:])
            pbc = res_pool.tile([32, 1], F32, name="pbc")
            pbb = res_pool.tile([32, 1], F32, name="pbb")
            nc.sync.dma_start(pbc[:20], pbc_d[:])
            nc.sync.dma_start(pbb[:5], pbb_d[:])

            # Level views. p3: feat in pad0[0:N3], scratch pad1/pad2 (full).
            # p4/p5: feat prefetched into pad0[N3:], scratch carved from
            # pad1/pad2 low regions.
            v3 = [_pad_view(pad0, 0, 64, 64),
                  _pad_view(pad1, 0, 64, 64),
                  _pad_view(pad2, 0, 64, 64)]
            v4 = [_pad_view(pad0, N3, 32, 32),
                  _pad_view(pad1, 0, 32, 32),
                  _pad_view(pad2, 0, 32, 32)]
            v5 = [_pad_view(pad0, N3 + N4, 16, 16),
                  _pad_view(pad1, N4, 16, 16),
                  _pad_view(pad2, N4, 16, 16)]

            # Prefetch all feats + zero all feat rings up front.
            for vs, lv, xd in zip((v3, v4, v5), LEVELS, x_d):
                _zero_ring(nc, vs[0], lv['H'], lv['W'])
                for c in range(NCH):
                    nc.sync.dma_start(
                        vs[0][:, c, 1:lv['H'] + 1, 1:lv['W'] + 1], xd[c])
            # p3 scratch rings
            for vs in (v3,):
                _zero_ring(nc, vs[1], 64, 64)
                _zero_ring(nc, vs[2], 64, 64)

            _pass(nc, psum_pool, wts_pool, stage_pool,
                  [(v3, 64, 64, 8, 0)],
                  sw_d, sbias, pwc, pwb, pbc, pbb, out_d, "a")

            # p4/p5 scratch rings (after p3 stops reading pad1/pad2)
            for vs, lv in zip((v4, v5), LEVELS[1:]):
                _zero_ring(nc, vs[1], lv['H'], lv['W'])
                _zero_ring(nc, vs[2], lv['H'], lv['W'])

            _pass(nc, psum_pool, wts_pool, stage_pool,
                  [(v4, 32, 32, 16, 4096), (v5, 16, 16, 16, 5120)],
                  sw_d, sbias, pwc, pwb, pbc, pbb, out_d, "b")

    nc.compile()
    return nc


def _pack_stem_w(wcls, wbox):
    # [s, l, co, ci, ky, kx] -> [s, l, cip, cic, coc, tap, cop]
    w = np.stack([wcls, wbox]).reshape(2, NL, NCH, P, NCH, P, 3, 3)
    w = w.transpose(0, 1, 5, 4, 2, 6, 7, 3)
    return np.ascontiguousarray(w.reshape(2, NL, P, NCH, NCH, 9, P),
                                dtype=np.float32)


def _pack_pred_w(w):
    # [co, ci, ky, kx] -> [cip, cic, tap, co]
    n = w.shape[0]
    w = w.reshape(n, NCH, P, 3, 3).transpose(2, 1, 3, 4, 0)
    return np.ascontiguousarray(w.reshape(P, NCH, 9, n), dtype=np.float32)


def kernel(p3, p4, p5, stem_cls_w, stem_cls_b, stem_box_w, stem_box_b,
           pred_cls_w, pred_cls_b, pred_box_w, pred_box_b,
           pred_ctr_w, pred_ctr_b):
    if 'nc' not in _cached:
        _cached['nc'] = _build()
    nc = _cached['nc']

    B = p3.shape[0]
    sw = _pack_stem_w(np.asarray(stem_cls_w), np.asarray(stem_box_w))
    sb = np.ascontiguousarray(
        np.stack([stem_cls_b, stem_box_b]).reshape(2, NL, NCH, P, 1),
        dtype=np.float32)
    pwc = _pack_pred_w(np.asarray(pred_cls_w))
    pwb = _pack_pred_w(np.concatenate([pred_box_w, pred_ctr_w], axis=0))
    pbc = np.asarray(pred_cls_b, np.float32).reshape(20, 1)
    pbb = np.concatenate([pred_box_b, pred_ctr_b]).astype(np.float32).reshape(5, 1)

    shared = {"sw": sw, "sb": sb, "pwc": pwc, "pwb": pwb,
              "pbc": pbc, "pbb": pbb}
    xs = [np.asarray(p3, np.float32), np.asarray(p4, np.float32),
          np.asarray(p5, np.float32)]
    in_maps = []
    for b in range(B):
        m = dict(shared)
        for i, x in enumerate(xs):
            m[f"x{i}"] = np.ascontiguousarray(
                x[b].reshape(NCH, P, x.shape[2], x.shape[3]))
        in_maps.append(m)

    res = run_bass_kernel_spmd(nc, in_maps, core_ids=list(range(B)),
                               **_run_opts)
    _last['res'] = res
    out = np.stack([r["out"].T for r in res.results])
    return np.ascontiguousarray(out, dtype=np.float32)


# revision 8
# speedup vs baseline: 1.0317x; 1.0208x over previous
"""FCOS head (nn_FCOS_73787538145418) Trainium2 Bass kernel.

Sharding: data-parallel, one image per NeuronCore (B=8 across 8 cores),
weights replicated. Each core runs the identical SPMD NEFF over its image.

Per level (p3 64x64, p4 32x32, p5 16x16): two 4-layer 3x3 conv stems
(cls/box, 256ch + ReLU), then prediction convs (cls 20ch; box+ctr 5ch).
3x3 'same' convs = 18 PSUM-accumulated matmuls (2 ci chunks x 9 taps,
K=128) over spatially padded SBUF buffers, dtype float32r (full-rate fp32
on the PE). Levels p4+p5 run as one fused pass sharing stem-weight DMAs.
The two prediction convs are column-tiled into PE col-groups 0 and 1 and
run concurrently. Output is [25, 5376] channel-major per core; the host
transposes and stacks to (8, 5376, 25).
"""
import sys

if '/opt/trn_rl_repo' not in sys.path:
    sys.path.insert(0, '/opt/trn_rl_repo')

import numpy as np

import concourse.bass as bass
import concourse.mybir as mybir
from concourse import bacc
import concourse.tile as tile
from concourse.bass_utils import run_bass_kernel_spmd

P = 128
NCH = 2                 # 256 channels = 2 chunks of 128
C = 256
NL = 4                  # stem depth
# (H, W, rows-per-pixel-tile, flat-offset of feat buffer, pixel base)
LEVELS = [
    dict(H=64, W=64, R=8, pix=0),
    dict(H=32, W=32, R=16, pix=4096),
    dict(H=16, W=16, R=16, pix=5120),
]
NPIX_TOTAL = 5376
F32R = mybir.dt.float32r
F32 = mybir.dt.float32

_cached = {}
_run_opts = {}   # extra kwargs for run_bass_kernel_spmd (test harness: trace)
_last = {}       # last BassKernelResults (test harness reads exec_time_ns)


def _pad_view(flat_tile, off, H, W):
    n = NCH * (H + 2) * (W + 2)
    return flat_tile[:, off:off + n].rearrange(
        "p (c h w) -> p c h w", c=NCH, h=H + 2, w=W + 2)


def _zero_ring(nc, v, H, W):
    f = v.bitcast(F32)
    for c in range(NCH):
        nc.vector.memset(f[:, c, 0, :], 0.0)
        nc.vector.memset(f[:, c, H + 1, :], 0.0)
        nc.vector.memset(f[:, c, 1:H + 1, 0], 0.0)
        nc.vector.memset(f[:, c, 1:H + 1, W + 1], 0.0)


def _conv_layer(nc, psum_pool, wt, src, dst, bias_ap, H, W, R, tag):
    """3x3 same conv 256->256 + bias + relu between padded fp32r views."""
    n_tiles = H // R
    for o in range(NCH):
        pss = [
            psum_pool.tile([P, R, W], F32, tag="ps", name=f"ps_{tag}_{o}_{it}")
            for it in range(n_tiles)
        ]
        k = 0
        for c in range(NCH):
            for ky in range(3):
                for kx in range(3):
                    lhsT = wt[:, c, o, ky * 3 + kx, :]
                    for it in range(n_tiles):
                        r0 = it * R
                        rhs = src[:, c, r0 + ky:r0 + ky + R, kx:kx + W]
                        nc.tensor.matmul(pss[it][:], lhsT, rhs,
                                         start=(k == 0), stop=(k == 17))
                    k += 1
        for it in range(n_tiles):
            r0 = it * R
            nc.scalar.activation(dst[:, o, r0 + 1:r0 + 1 + R, 1:W + 1],
                                 pss[it][:],
                                 mybir.ActivationFunctionType.Relu,
                                 bias=bias_ap[:, o])


def _preds(nc, psum_pool, stage_pool, pwc, pwb, pbc, pbb,
           cls_tower, box_tower, out_d, H, W, R, pix_base, tag):
    """cls (20ch) and box+ctr (5ch) 3x3 prediction convs (PSUM base 0)."""
    n_tiles = H // R
    for it in range(n_tiles):
        r0 = it * R
        ps1 = psum_pool.tile([P, R, W], F32, tag="ps", name=f"pc_{tag}_{it}")
        ps2 = psum_pool.tile([P, R, W], F32, tag="ps", name=f"pb_{tag}_{it}")
        k = 0
        for c in range(NCH):
            for ky in range(3):
                for kx in range(3):
                    t = ky * 3 + kx
                    rc = cls_tower[:, c, r0 + ky:r0 + ky + R, kx:kx + W]
                    rb = box_tower[:, c, r0 + ky:r0 + ky + R, kx:kx + W]
                    nc.tensor.matmul(ps1[0:20], pwc[:, c, t, :], rc,
                                     start=(k == 0), stop=(k == 17))
                    nc.tensor.matmul(ps2[0:5], pwb[:, c, t, :], rb,
                                     start=(k == 0), stop=(k == 17))
                    k += 1
        st = stage_pool.tile([32, R * W], F32, tag="st", name=f"st_{tag}_{it}")
        st2 = stage_pool.tile([32, R * W], F32, tag="st", name=f"s2_{tag}_{it}")
        nc.vector.tensor_tensor(
            st[0:20], ps1[0:20].rearrange("p r w -> p (r w)"),
            pbc[:20].to_broadcast([20, R * W]), mybir.AluOpType.add)
        nc.vector.tensor_tensor(
            st2[0:5], ps2[0:5].rearrange("p r w -> p (r w)"),
            pbb[:5].to_broadcast([5, R * W]), mybir.AluOpType.add)
        c0 = pix_base + r0 * W
        nc.sync.dma_start(out_d[0:20, c0:c0 + R * W], st[0:20])
        nc.sync.dma_start(out_d[20:25, c0:c0 + R * W], st2[0:5])


# Buffer rotation (3 pad buffers v0=feat, v1, v2):
#   cls stem: v0->v1->v2->v1->v2   (cls tower = v2)
#   box stem: v0->v1->v0->v1->v0   (box tower = v0; feat dead after box l1)
_CLS_CHAIN = [(0, 1), (1, 2), (2, 1), (1, 2)]
_BOX_CHAIN = [(0, 1), (1, 0), (0, 1), (1, 0)]


def _pass(nc, psum_pool, wts_pool, stage_pool, lvl_views, sw_d, sbias,
          pwc, pwb, pbc, pbb, out_d, tag):
    """One full pass (stems + preds) over a list of levels sharing weight DMAs.

    lvl_views: list of (views[3], H, W, R, pix_base)."""
    for s in range(2):
        chain = _CLS_CHAIN if s == 0 else _BOX_CHAIN
        for l in range(NL):
            wt = wts_pool.tile([P, NCH, NCH, 9, P], F32R,
                               tag="w", name=f"w_{tag}_{s}_{l}")
            for c in range(NCH):
                for o in range(NCH):
                    nc.sync.dma_start(wt[:, c, o], sw_d[s, l, :, c, o])
            si, di = chain[l]
            for vi, (views, H, W, R, pix) in enumerate(lvl_views):
                _conv_layer(nc, psum_pool, wt, views[si], views[di],
                            sbias[:, s, l], H, W, R, f"{tag}{vi}_{s}{l}")
    for vi, (views, H, W, R, pix) in enumerate(lvl_views):
        _preds(nc, psum_pool, stage_pool, pwc, pwb, pbc, pbb,
               views[2], views[0], out_d, H, W, R, pix, f"{tag}{vi}")


def _build():
    nc = bacc.Bacc("TRN2", target_bir_lowering=False, debug=False,
                   num_devices=8)

    x_d = [nc.dram_tensor(f"x{i}", (NCH, P, lv['H'] + 2, lv['W'] + 2),
                          F32R, kind="ExternalInput")
           for i, lv in enumerate(LEVELS)]
    sw_d = nc.dram_tensor("sw", (2, NL, P, NCH, NCH, 9, P), F32R,
                          kind="ExternalInput")
    sb_d = nc.dram_tensor("sb", (2, NL, NCH, P, 1), F32, kind="ExternalInput")
    pwc_d = nc.dram_tensor("pwc", (P, NCH, 9, 20), F32R, kind="ExternalInput")
    pwb_d = nc.dram_tensor("pwb", (P, NCH, 9, 5), F32R, kind="ExternalInput")
    pbc_d = nc.dram_tensor("pbc", (20, 1), F32, kind="ExternalInput")
    pbb_d = nc.dram_tensor("pbb", (5, 1), F32, kind="ExternalInput")
    out_d = nc.dram_tensor("out", (25, NPIX_TOTAL), F32, kind="ExternalOutput")

    N3 = NCH * 66 * 66            # 8712: p3 padded elems/partition
    N4 = NCH * 34 * 34            # 2312
    N5 = NCH * 18 * 18            # 648
    # pad0 additionally holds the p4/p5 feat regions (prefetched at start)
    PAD0 = N3 + N4 + N5

    with tile.TileContext(nc) as tc:
        with (
            tc.tile_pool(name="resident", bufs=1) as res_pool,
            tc.tile_pool(name="wts", bufs=3) as wts_pool,
            tc.tile_pool(name="psum", bufs=8, space="PSUM") as psum_pool,
            tc.tile_pool(name="stage", bufs=4) as stage_pool,
        ):
            pad0 = res_pool.tile([P, PAD0], F32R, name="pad0")
            pad1 = res_pool.tile([P, N3], F32R, name="pad1")
            pad2 = res_pool.tile([P, N3], F32R, name="pad2")

            sbias = res_pool.tile([P, 2, NL, NCH, 1], F32, name="sbias")
            nc.sync.dma_start(
                sbias[:],
                sb_d[:].rearrange("s l a p o -> p (s l a o)")
                       .rearrange("p (s l a o) -> p s l a o",
                                  s=2, l=NL, a=NCH))
            pwc = res_pool.tile([P, NCH, 9, 20], F32R, name="pwc")
            pwb = res_pool.tile([P, NCH, 9, 5], F32R, name="pwb")
            nc.sync.dma_start(pwc[:], pwc_d[:])
            nc.sync.dma_start(pwb[:], pwb_d[:])
            pbc = res_pool.tile([32, 1], F32, name="pbc")
            pbb = res_pool.tile([32, 1], F32, name="pbb")
            nc.sync.dma_start(pbc[:20], pbc_d[:])
            nc.sync.dma_start(pbb[:5], pbb_d[:])

            # Level views. p3: feat in pad0[0:N3], scratch pad1/pad2 (full).
            # p4/p5: feat prefetched into pad0[N3:], scratch carved from
            # pad1/pad2 low regions.
            v3 = [_pad_view(pad0, 0, 64, 64),
                  _pad_view(pad1, 0, 64, 64),
                  _pad_view(pad2, 0, 64, 64)]
            v4 = [_pad_view(pad0, N3, 32, 32),
                  _pad_view(pad1, 0, 32, 32),
                  _pad_view(pad2, 0, 32, 32)]
            v5 = [_pad_view(pad0, N3 + N4, 16, 16),
                  _pad_view(pad1, N4, 16, 16),
                  _pad_view(pad2, N4, 16, 16)]

            # Prefetch all feats up front (host ships them pre-padded,
            # so the transfers are contiguous and the rings arrive zeroed).
            for vs, lv, xd in zip((v3, v4, v5), LEVELS, x_d):
                for c in range(NCH):
                    nc.sync.dma_start(vs[0][:, c], xd[c])
            # p3 scratch rings
            for vs in (v3,):
                _zero_ring(nc, vs[1], 64, 64)
                _zero_ring(nc, vs[2], 64, 64)

            _pass(nc, psum_pool, wts_pool, stage_pool,
                  [(v3, 64, 64, 8, 0)],
                  sw_d, sbias, pwc, pwb, pbc, pbb, out_d, "a")

            # p4/p5 scratch rings (after p3 stops reading pad1/pad2)
            for vs, lv in zip((v4, v5), LEVELS[1:]):
                _zero_ring(nc, vs[1], lv['H'], lv['W'])
                _zero_ring(nc, vs[2], lv['H'], lv['W'])

            _pass(nc, psum_pool, wts_pool, stage_pool,
                  [(v4, 32, 32, 16, 4096), (v5, 16, 16, 16, 5120)],
                  sw_d, sbias, pwc, pwb, pbc, pbb, out_d, "b")

    nc.compile()
    return nc


def _pack_stem_w(wcls, wbox):
    # [s, l, co, ci, ky, kx] -> [s, l, cip, cic, coc, tap, cop]
    w = np.stack([wcls, wbox]).reshape(2, NL, NCH, P, NCH, P, 3, 3)
    w = w.transpose(0, 1, 5, 4, 2, 6, 7, 3)
    return np.ascontiguousarray(w.reshape(2, NL, P, NCH, NCH, 9, P),
                                dtype=np.float32)


def _pack_pred_w(w):
    # [co, ci, ky, kx] -> [cip, cic, tap, co]
    n = w.shape[0]
    w = w.reshape(n, NCH, P, 3, 3).transpose(2, 1, 3, 4, 0)
    return np.ascontiguousarray(w.reshape(P, NCH, 9, n), dtype=np.float32)


def kernel(p3, p4, p5, stem_cls_w, stem_cls_b, stem_box_w, stem_box_b,
           pred_cls_w, pred_cls_b, pred_box_w, pred_box_b,
           pred_ctr_w, pred_ctr_b):
    if 'nc' not in _cached:
        _cached['nc'] = _build()
    nc = _cached['nc']

    B = p3.shape[0]
    sw = _pack_stem_w(np.asarray(stem_cls_w), np.asarray(stem_box_w))
    sb = np.ascontiguousarray(
        np.stack([stem_cls_b, stem_box_b]).reshape(2, NL, NCH, P, 1),
        dtype=np.float32)
    pwc = _pack_pred_w(np.asarray(pred_cls_w))
    pwb = _pack_pred_w(np.concatenate([pred_box_w, pred_ctr_w], axis=0))
    pbc = np.asarray(pred_cls_b, np.float32).reshape(20, 1)
    pbb = np.concatenate([pred_box_b, pred_ctr_b]).astype(np.float32).reshape(5, 1)

    shared = {"sw": sw, "sb": sb, "pwc": pwc, "pwb": pwb,
              "pbc": pbc, "pbb": pbb}
    xs = [np.asarray(p3, np.float32), np.asarray(p4, np.float32),
          np.asarray(p5, np.float32)]
    in_maps = []
    for b in range(B):
        m = dict(shared)
        for i, x in enumerate(xs):
            m[f"x{i}"] = np.pad(
                x[b].reshape(NCH, P, x.shape[2], x.shape[3]),
                ((0, 0), (0, 0), (1, 1), (1, 1)))
        in_maps.append(m)

    res = run_bass_kernel_spmd(nc, in_maps, core_ids=list(range(B)),
                               **_run_opts)
    _last['res'] = res
    out = np.stack([r["out"].T for r in res.results])
    return np.ascontiguousarray(out, dtype=np.float32)


# revision 9
# speedup vs baseline: 1.0329x; 1.0012x over previous
"""FCOS head (nn_FCOS_73787538145418) Trainium2 Bass kernel.

Sharding: data-parallel, one image per NeuronCore (B=8 across 8 cores),
weights replicated. Each core runs the identical SPMD NEFF over its image.

Per level (p3 64x64, p4 32x32, p5 16x16): two 4-layer 3x3 conv stems
(cls/box, 256ch + ReLU), then prediction convs (cls 20ch; box+ctr 5ch).
3x3 'same' convs = 18 PSUM-accumulated matmuls (2 ci chunks x 9 taps,
K=128) over spatially padded SBUF buffers, dtype float32r (full-rate fp32
on the PE). Levels p4+p5 run as one fused pass sharing stem-weight DMAs.
The two prediction convs are column-tiled into PE col-groups 0 and 1 and
run concurrently. Output is [25, 5376] channel-major per core; the host
transposes and stacks to (8, 5376, 25).
"""
import sys

if '/opt/trn_rl_repo' not in sys.path:
    sys.path.insert(0, '/opt/trn_rl_repo')

import numpy as np

import concourse.bass as bass
import concourse.mybir as mybir
from concourse import bacc
import concourse.tile as tile
from concourse.bass_utils import run_bass_kernel_spmd

P = 128
NCH = 2                 # 256 channels = 2 chunks of 128
C = 256
NL = 4                  # stem depth
# (H, W, rows-per-pixel-tile, flat-offset of feat buffer, pixel base)
LEVELS = [
    dict(H=64, W=64, R=8, pix=0),
    dict(H=32, W=32, R=16, pix=4096),
    dict(H=16, W=16, R=16, pix=5120),
]
NPIX_TOTAL = 5376
F32R = mybir.dt.float32r
F32 = mybir.dt.float32

_cached = {}
_run_opts = {}   # extra kwargs for run_bass_kernel_spmd (test harness: trace)
_last = {}       # last BassKernelResults (test harness reads exec_time_ns)


def _pad_view(flat_tile, off, H, W):
    n = NCH * (H + 2) * (W + 2)
    return flat_tile[:, off:off + n].rearrange(
        "p (c h w) -> p c h w", c=NCH, h=H + 2, w=W + 2)


def _zero_ring(nc, v, H, W):
    f = v.bitcast(F32)
    for c in range(NCH):
        nc.vector.memset(f[:, c, 0, :], 0.0)
        nc.vector.memset(f[:, c, H + 1, :], 0.0)
        nc.vector.memset(f[:, c, 1:H + 1, 0], 0.0)
        nc.vector.memset(f[:, c, 1:H + 1, W + 1], 0.0)


def _conv_layer(nc, psum_pool, wt, src, dst, bias_ap, H, W, R, tag):
    """3x3 same conv 256->256 + bias + relu between padded fp32r views."""
    n_tiles = H // R
    for o in range(NCH):
        pss = [
            psum_pool.tile([P, R, W], F32, tag="ps", name=f"ps_{tag}_{o}_{it}")
            for it in range(n_tiles)
        ]
        k = 0
        for c in range(NCH):
            for ky in range(3):
                for kx in range(3):
                    lhsT = wt[:, c, o, ky * 3 + kx, :]
                    for it in range(n_tiles):
                        r0 = it * R
                        rhs = src[:, c, r0 + ky:r0 + ky + R, kx:kx + W]
                        nc.tensor.matmul(pss[it][:], lhsT, rhs,
                                         start=(k == 0), stop=(k == 17))
                    k += 1
        for it in range(n_tiles):
            r0 = it * R
            nc.scalar.activation(dst[:, o, r0 + 1:r0 + 1 + R, 1:W + 1],
                                 pss[it][:],
                                 mybir.ActivationFunctionType.Relu,
                                 bias=bias_ap[:, o])


def _preds(nc, psum_pool, stage_pool, pwc, pwb, pbc, pbb,
           cls_tower, box_tower, out_d, H, W, R, pix_base, tag):
    """cls (20ch) and box+ctr (5ch) 3x3 prediction convs (PSUM base 0)."""
    n_tiles = H // R
    for it in range(n_tiles):
        r0 = it * R
        ps1 = psum_pool.tile([P, R, W], F32, tag="ps", name=f"pc_{tag}_{it}")
        ps2 = psum_pool.tile([P, R, W], F32, tag="ps", name=f"pb_{tag}_{it}")
        k = 0
        for c in range(NCH):
            for ky in range(3):
                for kx in range(3):
                    t = ky * 3 + kx
                    rc = cls_tower[:, c, r0 + ky:r0 + ky + R, kx:kx + W]
                    rb = box_tower[:, c, r0 + ky:r0 + ky + R, kx:kx + W]
                    nc.tensor.matmul(ps1[0:20], pwc[:, c, t, :], rc,
                                     start=(k == 0), stop=(k == 17))
                    nc.tensor.matmul(ps2[0:5], pwb[:, c, t, :], rb,
                                     start=(k == 0), stop=(k == 17))
                    k += 1
        st = stage_pool.tile([32, R * W], F32, tag="st", name=f"st_{tag}_{it}")
        st2 = stage_pool.tile([32, R * W], F32, tag="st", name=f"s2_{tag}_{it}")
        nc.vector.tensor_tensor(
            st[0:20], ps1[0:20].rearrange("p r w -> p (r w)"),
            pbc[:20].to_broadcast([20, R * W]), mybir.AluOpType.add)
        nc.vector.tensor_tensor(
            st2[0:5], ps2[0:5].rearrange("p r w -> p (r w)"),
            pbb[:5].to_broadcast([5, R * W]), mybir.AluOpType.add)
        c0 = pix_base + r0 * W
        nc.sync.dma_start(out_d[0:20, c0:c0 + R * W], st[0:20])
        nc.sync.dma_start(out_d[20:25, c0:c0 + R * W], st2[0:5])


# Buffer rotation (3 pad buffers v0=feat, v1, v2):
#   cls stem: v0->v1->v2->v1->v2   (cls tower = v2)
#   box stem: v0->v1->v0->v1->v0   (box tower = v0; feat dead after box l1)
_CLS_CHAIN = [(0, 1), (1, 2), (2, 1), (1, 2)]
_BOX_CHAIN = [(0, 1), (1, 0), (0, 1), (1, 0)]


def _pass(nc, psum_pool, wts_pool, stage_pool, lvl_views, sw_d, sbias,
          pwc, pwb, pbc, pbb, out_d, tag):
    """One full pass (stems + preds) over a list of levels sharing weight DMAs.

    lvl_views: list of (views[3], H, W, R, pix_base)."""
    for s in range(2):
        chain = _CLS_CHAIN if s == 0 else _BOX_CHAIN
        for l in range(NL):
            wt = wts_pool.tile([P, NCH, NCH, 9, P], F32R,
                               tag="w", name=f"w_{tag}_{s}_{l}")
            for c in range(NCH):
                for o in range(NCH):
                    nc.sync.dma_start(wt[:, c, o], sw_d[s, l, :, c, o])
            si, di = chain[l]
            for vi, (views, H, W, R, pix) in enumerate(lvl_views):
                _conv_layer(nc, psum_pool, wt, views[si], views[di],
                            sbias[:, s, l], H, W, R, f"{tag}{vi}_{s}{l}")
    for vi, (views, H, W, R, pix) in enumerate(lvl_views):
        _preds(nc, psum_pool, stage_pool, pwc, pwb, pbc, pbb,
               views[2], views[0], out_d, H, W, R, pix, f"{tag}{vi}")


def _build():
    nc = bacc.Bacc("TRN2", target_bir_lowering=False, debug=False,
                   num_devices=8)

    x_d = [nc.dram_tensor(f"x{i}", (NCH, P, lv['H'] + 2, lv['W'] + 2),
                          F32R, kind="ExternalInput")
           for i, lv in enumerate(LEVELS)]
    sw_d = nc.dram_tensor("sw", (2, NL, P, NCH, NCH, 9, P), F32R,
                          kind="ExternalInput")
    sb_d = nc.dram_tensor("sb", (2, NL, NCH, P, 1), F32, kind="ExternalInput")
    pwc_d = nc.dram_tensor("pwc", (P, NCH, 9, 20), F32R, kind="ExternalInput")
    pwb_d = nc.dram_tensor("pwb", (P, NCH, 9, 5), F32R, kind="ExternalInput")
    pbc_d = nc.dram_tensor("pbc", (20, 1), F32, kind="ExternalInput")
    pbb_d = nc.dram_tensor("pbb", (5, 1), F32, kind="ExternalInput")
    out_d = nc.dram_tensor("out", (25, NPIX_TOTAL), F32, kind="ExternalOutput")

    N3 = NCH * 66 * 66            # 8712: p3 padded elems/partition
    N4 = NCH * 34 * 34            # 2312
    N5 = NCH * 18 * 18            # 648
    # pad0 additionally holds the p4/p5 feat regions (prefetched at start)
    PAD0 = N3 + N4 + N5

    with tile.TileContext(nc) as tc:
        with (
            tc.tile_pool(name="resident", bufs=1) as res_pool,
            tc.tile_pool(name="wts", bufs=3) as wts_pool,
            tc.tile_pool(name="psum", bufs=8, space="PSUM") as psum_pool,
            tc.tile_pool(name="stage", bufs=4) as stage_pool,
        ):
            pad0 = res_pool.tile([P, PAD0], F32R, name="pad0")
            pad1 = res_pool.tile([P, N3], F32R, name="pad1")
            pad2 = res_pool.tile([P, N3], F32R, name="pad2")

            sbias = res_pool.tile([P, 2, NL, NCH, 1], F32, name="sbias")
            nc.sync.dma_start(
                sbias[:],
                sb_d[:].rearrange("s l a p o -> p (s l a o)")
                       .rearrange("p (s l a o) -> p s l a o",
                                  s=2, l=NL, a=NCH))
            pwc = res_pool.tile([P, NCH, 9, 20], F32R, name="pwc")
            pwb = res_pool.tile([P, NCH, 9, 5], F32R, name="pwb")
            nc.sync.dma_start(pwc[:], pwc_d[:])
            nc.sync.dma_start(pwb[:], pwb_d[:])
            pbc = res_pool.tile([32, 1], F32, name="pbc")
            pbb = res_pool.tile([32, 1], F32, name="pbb")
            nc.sync.dma_start(pbc[:20], pbc_d[:])
            nc.sync.dma_start(pbb[:5], pbb_d[:])

            # Level views. p3: feat in pad0[0:N3], scratch pad1/pad2 (full).
            # p4/p5: feat prefetched into pad0[N3:], scratch carved from
            # pad1/pad2 low regions.
            v3 = [_pad_view(pad0, 0, 64, 64),
                  _pad_view(pad1, 0, 64, 64),
                  _pad_view(pad2, 0, 64, 64)]
            v4 = [_pad_view(pad0, N3, 32, 32),
                  _pad_view(pad1, 0, 32, 32),
                  _pad_view(pad2, 0, 32, 32)]
            v5 = [_pad_view(pad0, N3 + N4, 16, 16),
                  _pad_view(pad1, N4, 16, 16),
                  _pad_view(pad2, N4, 16, 16)]

            # Prefetch all feats up front (host ships them pre-padded,
            # so the transfers are contiguous and the rings arrive zeroed).
            # p3 feat is split into row bands so the first conv tiles' deps
            # clear after the first band, not the whole 4MB image.
            for c in range(NCH):
                for b0 in range(0, 66, 11):
                    nc.sync.dma_start(v3[0][:, c, b0:b0 + 11], x_d[0][c, :, b0:b0 + 11])
            for vs, xd in ((v4, x_d[1]), (v5, x_d[2])):
                for c in range(NCH):
                    nc.sync.dma_start(vs[0][:, c], xd[c])
            # p3 scratch rings
            for vs in (v3,):
                _zero_ring(nc, vs[1], 64, 64)
                _zero_ring(nc, vs[2], 64, 64)

            _pass(nc, psum_pool, wts_pool, stage_pool,
                  [(v3, 64, 64, 8, 0)],
                  sw_d, sbias, pwc, pwb, pbc, pbb, out_d, "a")

            # p4/p5 scratch rings (after p3 stops reading pad1/pad2)
            for vs, lv in zip((v4, v5), LEVELS[1:]):
                _zero_ring(nc, vs[1], lv['H'], lv['W'])
                _zero_ring(nc, vs[2], lv['H'], lv['W'])

            _pass(nc, psum_pool, wts_pool, stage_pool,
                  [(v4, 32, 32, 16, 4096), (v5, 16, 16, 16, 5120)],
                  sw_d, sbias, pwc, pwb, pbc, pbb, out_d, "b")

    nc.compile()
    return nc


def _pack_stem_w(wcls, wbox):
    # [s, l, co, ci, ky, kx] -> [s, l, cip, cic, coc, tap, cop]
    w = np.stack([wcls, wbox]).reshape(2, NL, NCH, P, NCH, P, 3, 3)
    w = w.transpose(0, 1, 5, 4, 2, 6, 7, 3)
    return np.ascontiguousarray(w.reshape(2, NL, P, NCH, NCH, 9, P),
                                dtype=np.float32)


def _pack_pred_w(w):
    # [co, ci, ky, kx] -> [cip, cic, tap, co]
    n = w.shape[0]
    w = w.reshape(n, NCH, P, 3, 3).transpose(2, 1, 3, 4, 0)
    return np.ascontiguousarray(w.reshape(P, NCH, 9, n), dtype=np.float32)


def kernel(p3, p4, p5, stem_cls_w, stem_cls_b, stem_box_w, stem_box_b,
           pred_cls_w, pred_cls_b, pred_box_w, pred_box_b,
           pred_ctr_w, pred_ctr_b):
    if 'nc' not in _cached:
        _cached['nc'] = _build()
    nc = _cached['nc']

    B = p3.shape[0]
    sw = _pack_stem_w(np.asarray(stem_cls_w), np.asarray(stem_box_w))
    sb = np.ascontiguousarray(
        np.stack([stem_cls_b, stem_box_b]).reshape(2, NL, NCH, P, 1),
        dtype=np.float32)
    pwc = _pack_pred_w(np.asarray(pred_cls_w))
    pwb = _pack_pred_w(np.concatenate([pred_box_w, pred_ctr_w], axis=0))
    pbc = np.asarray(pred_cls_b, np.float32).reshape(20, 1)
    pbb = np.concatenate([pred_box_b, pred_ctr_b]).astype(np.float32).reshape(5, 1)

    shared = {"sw": sw, "sb": sb, "pwc": pwc, "pwb": pwb,
              "pbc": pbc, "pbb": pbb}
    xs = [np.asarray(p3, np.float32), np.asarray(p4, np.float32),
          np.asarray(p5, np.float32)]
    in_maps = []
    for b in range(B):
        m = dict(shared)
        for i, x in enumerate(xs):
            m[f"x{i}"] = np.pad(
                x[b].reshape(NCH, P, x.shape[2], x.shape[3]),
                ((0, 0), (0, 0), (1, 1), (1, 1)))
        in_maps.append(m)

    res = run_bass_kernel_spmd(nc, in_maps, core_ids=list(range(B)),
                               **_run_opts)
    _last['res'] = res
    out = np.stack([r["out"].T for r in res.results])
    return np.ascontiguousarray(out, dtype=np.float32)
